# revision 26
# baseline (speedup 1.0000x reference)
"""VQ codebook-lookup kernel for Trainium2 (Bass/Tile), 8-core data-parallel.

Problem: z [16, 4096, 512] f32, codebook e [1024, 512] f32.
The reference computes d[n,k] = -|z_n|^2 - |e_k|^2 + 2 z_n.e_k in fp32, where
the |z|^2 term (~512) quantizes d at ulp(512) ~ 6e-5 — hundreds of rows have
true gaps below that, so argmax ties are resolved by this exact quantization.
We replicate the same fp32 arithmetic shape (verified bit-robust offline):
  negd[n,k] = fl( fl(|e_k|^2 + |z_n|^2) - (2 z_n).e_k )     (= -d)
  idx[n]    = argmin_k negd  (first occurrence, via max_index equality search)
  z_q       = e[idx]          (z_q_st == z_q numerically)
  loss      = 1.25 * (sum_n min_k negd) / (N * D)    since |z-e|^2 = -d
The factor 2 is folded into the codebook on the host (exact fp32 scaling).

Sharding: z split into 8 shards of 8192 rows (batch-dim data parallel),
codebook replicated. Per-core kernel computes its z_q shard, indices shard
and the scalar partial sum; host sums partials into the loss.
"""

import numpy as np

import concourse.bacc as bacc
import concourse.bass as bass
import concourse.mybir as mybir
import concourse.tile as tile
from concourse import bass_utils
from concourse.bass import IndirectOffsetOnAxis
from concourse.masks import make_identity

F32 = mybir.dt.float32
BF16 = mybir.dt.bfloat16
U32 = mybir.dt.uint32

P = 128          # partitions / rows per tile
D = 512          # embedding dim
K = 1024         # number of codes
N_CORES = 8
ROWS_PER_CORE = 8192


def build_core_program(n_tiles=ROWS_PER_CORE // P):
    """Bass program for one core's shard: [n_tiles*128, 512] rows of z."""
    rows = n_tiles * P
    nc = bacc.Bacc(None, target_bir_lowering=False)

    z_d = nc.dram_tensor("z", [rows, D], F32, kind="ExternalInput")
    ehT_d = nc.dram_tensor("ehT", [D, K], BF16, kind="ExternalInput")
    elT_d = nc.dram_tensor("elT", [D, K], BF16, kind="ExternalInput")
    e_d = nc.dram_tensor("e", [K, D], F32, kind="ExternalInput")
    bias_d = nc.dram_tensor("bias", [P, K], F32, kind="ExternalInput")

    zq_d = nc.dram_tensor("zq", [rows, D], F32, kind="ExternalOutput")
    idx_d = nc.dram_tensor("idx", [P, n_tiles], U32, kind="ExternalOutput")
    part_d = nc.dram_tensor("partial", [P, 1], F32, kind="ExternalOutput")

    with tile.TileContext(nc) as tc:
        with (
            tc.tile_pool(name="const", bufs=1) as cpool,
            tc.tile_pool(name="zin", bufs=4) as zpool,
            tc.tile_pool(name="zn", bufs=3) as znpool,
            tc.tile_pool(name="zt", bufs=3) as ztpool,
            tc.tile_pool(name="sc", bufs=3) as scpool,
            tc.tile_pool(name="zq", bufs=3) as zqpool,
            tc.tile_pool(name="sq", bufs=2) as sqpool,
            tc.tile_pool(name="idx8", bufs=4) as ipool,
            tc.tile_pool(name="ps_sc", bufs=4, space="PSUM") as ps_sc,
        ):
            # ---- one-time setup ----
            ehT_sb = cpool.tile([P, 4, K], BF16)   # [p, dchunk, k]
            nc.sync.dma_start(
                out=ehT_sb[:], in_=ehT_d[:].rearrange("(c p) k -> p c k", p=P)
            )
            elT_sb = cpool.tile([P, 4, K], BF16)
            nc.sync.dma_start(
                out=elT_sb[:], in_=elT_d[:].rearrange("(c p) k -> p c k", p=P)
            )
            bias_sb = cpool.tile([P, K], F32)
            nc.sync.dma_start(out=bias_sb[:], in_=bias_d[:])

            mslots = cpool.tile([P, n_tiles], F32)
            zslots = cpool.tile([P, n_tiles], F32)
            idxc = cpool.tile([P, n_tiles], U32)

            # ---- main loop over 128-row tiles ----
            for t in range(n_tiles):
                z_sb = zpool.tile([P, D], F32)
                nc.sync.dma_start(out=z_sb[:], in_=z_d[t * P : (t + 1) * P, :])

                # bf16 hi/lo split in normal layout (Dekker-style):
                # hi-cast on the scalar engine, residual on DVE
                zh_n = znpool.tile([P, D], BF16)
                nc.scalar.activation(
                    out=zh_n[:], in_=z_sb[:], func=mybir.ActivationFunctionType.Copy
                )
                zl_n = znpool.tile([P, D], BF16)
                nc.vector.tensor_tensor(
                    out=zl_n[:], in0=z_sb[:], in1=zh_n[:], op=mybir.AluOpType.subtract
                )
                # transpose both bf16 operands via the DMA xbar (no PE time)
                zhT = ztpool.tile([P, D], BF16)
                zlT = ztpool.tile([P, D], BF16)
                for dc in range(4):
                    cs = slice(dc * P, (dc + 1) * P)
                    nc.sync.dma_start(out=zhT[:, cs], in_=zh_n[:, cs], transpose=True)
                    nc.sync.dma_start(out=zlT[:, cs], in_=zl_n[:, cs], transpose=True)

                # row norms: zslots[:, t] = sum_d z^2 (per partition row)
                zsq = sqpool.tile([P, D], F32)
                nc.scalar.activation(
                    out=zsq[:],
                    in_=z_sb[:],
                    func=mybir.ActivationFunctionType.Square,
                    accum_out=zslots[:, t : t + 1],
                )

                # 2c = zh.ehT + zh.elT + zl.ehT (bf16x3), PSUM f32 accumulate.
                # All zh passes first (zh is ready one DVE op earlier than zl),
                # weights reused across kh to cut LDWEIGHTS pressure.
                sc_ps = ps_sc.tile([P, K], F32, space="PSUM")
                for dc in range(4):
                    zh_c = zhT[:, dc * P : (dc + 1) * P]
                    for kh in range(2):
                        ks = slice(kh * 512, (kh + 1) * 512)
                        nc.tensor.matmul(
                            out=sc_ps[:, ks], lhsT=zh_c, rhs=ehT_sb[:, dc, ks],
                            start=(dc == 0), stop=False,
                        )
                        nc.tensor.matmul(
                            out=sc_ps[:, ks], lhsT=zh_c, rhs=elT_sb[:, dc, ks],
                            start=False, stop=False,
                        )
                for dc in range(4):
                    zl_c = zlT[:, dc * P : (dc + 1) * P]
                    for kh in range(2):
                        ks = slice(kh * 512, (kh + 1) * 512)
                        nc.tensor.matmul(
                            out=sc_ps[:, ks], lhsT=zl_c, rhs=ehT_sb[:, dc, ks],
                            start=False, stop=(dc == 3),
                        )

                # negd = fl(fl(|e_k|^2 + A_n) - 2c)  — mimics reference rounding
                sc_sb = scpool.tile([P, K], F32)
                nc.vector.scalar_tensor_tensor(
                    out=sc_sb[:],
                    in0=bias_sb[:],
                    scalar=zslots[:, t : t + 1],
                    in1=sc_ps[:],
                    op0=mybir.AluOpType.add,
                    op1=mybir.AluOpType.subtract,
                )
                # row min = |z_n - e_idx|^2 (quantized), also loss contribution
                nc.vector.tensor_reduce(
                    out=mslots[:, t : t + 1],
                    in_=sc_sb[:],
                    axis=mybir.AxisListType.X,
                    op=mybir.AluOpType.min,
                )
                # argmin: first position equal to the row min
                idx8 = ipool.tile([P, 8], U32)
                nc.vector.max_index(
                    out=idx8[:],
                    in_max=mslots[:, t : t + 1].to_broadcast([P, 8]),
                    in_values=sc_sb[:],
                )
                nc.gpsimd.tensor_copy(idxc[:, t : t + 1], idx8[:, 0:1])

                # gather z_q rows from the codebook in DRAM
                zq_sb = zqpool.tile([P, D], F32)
                nc.gpsimd.indirect_dma_start(
                    out=zq_sb[:],
                    out_offset=None,
                    in_=e_d[:],
                    in_offset=IndirectOffsetOnAxis(ap=idx8[:, 0:1], axis=0),
                )
                # faithful straight-through: out = fl(z + fl(z_q - z))
                dd_sb = zqpool.tile([P, D], F32)
                nc.gpsimd.tensor_tensor(
                    out=dd_sb[:], in0=zq_sb[:], in1=z_sb[:],
                    op=mybir.AluOpType.subtract,
                )
                st_sb = zqpool.tile([P, D], F32)
                nc.gpsimd.tensor_tensor(
                    out=st_sb[:], in0=z_sb[:], in1=dd_sb[:], op=mybir.AluOpType.add
                )
                nc.sync.dma_start(out=zq_d[t * P : (t + 1) * P, :], in_=st_sb[:])

            # ---- tail: loss partial + indices out (host sums partials) ----
            msum = cpool.tile([P, 1], F32)
            nc.vector.reduce_sum(msum[:], mslots[:], axis=mybir.AxisListType.X)
            nc.sync.dma_start(out=part_d[:], in_=msum[:])
            nc.sync.dma_start(out=idx_d[:], in_=idxc[:])

    nc.compile()
    return nc


_NC_CACHE = {}


def _get_nc(n_tiles):
    if n_tiles not in _NC_CACHE:
        _NC_CACHE[n_tiles] = build_core_program(n_tiles)
    return _NC_CACHE[n_tiles]


def kernel(z, embed_weight, _trace=False):
    z = np.asarray(z, dtype=np.float32)
    e = np.ascontiguousarray(np.asarray(embed_weight, dtype=np.float32))
    B, T, Dd = z.shape
    assert Dd == D and e.shape == (K, D)
    n = B * T
    zf = np.ascontiguousarray(z.reshape(n, D))
    rows = n // N_CORES
    n_tiles = rows // P

    import ml_dtypes

    eT2 = 2.0 * e.T.astype(np.float32)                 # [512, 1024], 2x exact
    ehT = eT2.astype(ml_dtypes.bfloat16)
    elT = (eT2 - ehT.astype(np.float32)).astype(ml_dtypes.bfloat16)
    ehT = np.ascontiguousarray(ehT)
    elT = np.ascontiguousarray(elT)
    bias_row = (e * e).sum(axis=1, dtype=np.float32)
    bias = np.ascontiguousarray(np.broadcast_to(bias_row[None, :], (P, K)))

    nc = _get_nc(n_tiles)
    in_maps = [
        {
            "z": zf[c * rows : (c + 1) * rows],
            "ehT": ehT,
            "elT": elT,
            "e": e,
            "bias": bias,
        }
        for c in range(N_CORES)
    ]
    res = bass_utils.run_bass_kernel_spmd(
        nc, in_maps, core_ids=list(range(N_CORES)), trace=_trace
    )

    zq = np.concatenate([res.results[c]["zq"] for c in range(N_CORES)], axis=0)
    z_q_st = zq.reshape(B, T, D)

    idx_parts = []
    for c in range(N_CORES):
        a = res.results[c]["idx"]                        # [128, n_tiles]
        idx_parts.append(np.ascontiguousarray(a.T).reshape(-1))  # row t*128+p
    indices = np.concatenate(idx_parts).astype(np.int32).reshape(B, T)

    total = np.sum(
        [res.results[c]["partial"].astype(np.float64).sum() for c in range(N_CORES)]
    )
    loss = np.float32(1.25 * total / (n * D))

    if _trace:
        kernel.last_exec_time_ns = res.exec_time_ns
        kernel.last_results = res
    return z_q_st, loss, indices


# revision 29
# speedup vs baseline: 2.1823x; 2.1823x over previous
"""VQ codebook-lookup kernel for Trainium2 (Bass/Tile), 8-core data-parallel.

Problem: z [16, 4096, 512] f32, codebook e [1024, 512] f32.
The reference computes d[n,k] = -|z_n|^2 - |e_k|^2 + 2 z_n.e_k in fp32, where
the |z|^2 term (~512) quantizes d at ulp(512) ~ 6e-5 — hundreds of rows have
true gaps below that, so argmax ties are resolved by this exact quantization.
We replicate the same fp32 arithmetic shape (verified bit-robust offline):
  negd[n,k] = fl( fl(|e_k|^2 + |z_n|^2) - (2 z_n).e_k )     (= -d)
  idx[n]    = argmin_k negd  (first occurrence, via max_index equality search)
  z_q       = e[idx]          (z_q_st == z_q numerically)
  loss      = 1.25 * (sum_n min_k negd) / (N * D)    since |z-e|^2 = -d
The factor 2 is folded into the codebook on the host (exact fp32 scaling).

Sharding: z split into 8 shards of 8192 rows (batch-dim data parallel),
codebook replicated. Per-core kernel computes its z_q shard, indices shard
and the scalar partial sum; host sums partials into the loss.
"""

import numpy as np

import concourse.bacc as bacc
import concourse.bass as bass
import concourse.mybir as mybir
import concourse.tile as tile
from concourse import bass_utils
from concourse.bass import IndirectOffsetOnAxis
from concourse.masks import make_identity

F32 = mybir.dt.float32
BF16 = mybir.dt.bfloat16
U32 = mybir.dt.uint32

P = 128          # partitions / rows per tile
D = 512          # embedding dim
K = 1024         # number of codes
N_CORES = 8
ROWS_PER_CORE = 8192


def build_core_program(n_tiles=ROWS_PER_CORE // P):
    """Bass program for one core's shard: [n_tiles*128, 512] rows of z."""
    rows = n_tiles * P
    nc = bacc.Bacc(None, target_bir_lowering=False)

    z_d = nc.dram_tensor("z", [rows, D], F32, kind="ExternalInput")
    ehT_d = nc.dram_tensor("ehT", [D, K], BF16, kind="ExternalInput")
    elT_d = nc.dram_tensor("elT", [D, K], BF16, kind="ExternalInput")
    e_d = nc.dram_tensor("e", [K, D], F32, kind="ExternalInput")
    bias_d = nc.dram_tensor("bias", [P, K], F32, kind="ExternalInput")

    zq_d = nc.dram_tensor("zq", [rows, D], F32, kind="ExternalOutput")
    idx_d = nc.dram_tensor("idx", [P, n_tiles], U32, kind="ExternalOutput")
    part_d = nc.dram_tensor("partial", [P, 1], F32, kind="ExternalOutput")

    with tile.TileContext(nc) as tc:
        with (
            tc.tile_pool(name="const", bufs=1) as cpool,
            tc.tile_pool(name="zin", bufs=4) as zpool,
            tc.tile_pool(name="zt", bufs=3) as ztpool,
            tc.tile_pool(name="sc", bufs=3) as scpool,
            tc.tile_pool(name="zq", bufs=3) as zqpool,
            tc.tile_pool(name="sq", bufs=2) as sqpool,
            tc.tile_pool(name="idx8", bufs=4) as ipool,
            tc.tile_pool(name="mh", bufs=4) as mhpool,
            tc.tile_pool(name="ps_zt", bufs=2, space="PSUM") as ps_zt,
            tc.tile_pool(name="ps_sc", bufs=6, space="PSUM") as ps_sc,
        ):
            # ---- one-time setup ----
            ident = cpool.tile([P, P], F32)
            make_identity(nc, ident[:])

            ehT_sb = cpool.tile([P, 4, K], BF16)   # [p, dchunk, k]
            nc.sync.dma_start(
                out=ehT_sb[:], in_=ehT_d[:].rearrange("(c p) k -> p c k", p=P)
            )
            elT_sb = cpool.tile([P, 4, K], BF16)
            nc.sync.dma_start(
                out=elT_sb[:], in_=elT_d[:].rearrange("(c p) k -> p c k", p=P)
            )
            bias_sb = cpool.tile([P, K], F32)
            nc.sync.dma_start(out=bias_sb[:], in_=bias_d[:])

            mslots = cpool.tile([P, n_tiles], F32)
            zslots = cpool.tile([P, n_tiles], F32)
            idxc = cpool.tile([P, n_tiles], U32)

            # ---- main loop over 128-row tiles ----
            for t in range(n_tiles):
                z_sb = zpool.tile([P, D], F32)
                nc.sync.dma_start(out=z_sb[:], in_=z_d[t * P : (t + 1) * P, :])

                # transpose z tile: zT[d, n] in 4 chunks of [128, 128]
                zT_ps = ps_zt.tile([P, D], F32, space="PSUM")
                for dc in range(4):
                    nc.tensor.transpose(
                        out=zT_ps[:, dc * P : (dc + 1) * P],
                        in_=z_sb[:, dc * P : (dc + 1) * P],
                        identity=ident[:],
                    )
                # bf16 hi/lo split of zT straight from PSUM (Dekker-style);
                # hi-cast on the (idle) scalar engine, residual on DVE
                zhT = ztpool.tile([P, D], BF16)
                nc.scalar.activation(
                    out=zhT[:], in_=zT_ps[:], func=mybir.ActivationFunctionType.Copy
                )
                zlT = ztpool.tile([P, D], BF16)
                nc.vector.tensor_tensor(
                    out=zlT[:], in0=zT_ps[:], in1=zhT[:], op=mybir.AluOpType.subtract
                )

                # row norms: zslots[:, t] = sum_d z^2 (per partition row)
                zsq = sqpool.tile([P, D], F32)
                nc.scalar.activation(
                    out=zsq[:],
                    in_=z_sb[:],
                    func=mybir.ActivationFunctionType.Square,
                    accum_out=zslots[:, t : t + 1],
                )

                # 2c = zh.ehT + zh.elT + zl.ehT (bf16x3), PSUM f32 accumulate.
                # One PSUM bank per k-half so banks free in half-tile quanta:
                # the DVE drain of half A overlaps half B's matmuls.
                sc_sb = scpool.tile([P, K], F32)
                mh = [None, None]
                for kh in range(2):
                    ks = slice(kh * 512, (kh + 1) * 512)
                    sc_half = ps_sc.tile([P, 512], F32, space="PSUM", tag="sc_half")
                    for dc in range(4):
                        zh_c = zhT[:, dc * P : (dc + 1) * P]
                        nc.tensor.matmul(
                            out=sc_half[:], lhsT=zh_c, rhs=ehT_sb[:, dc, ks],
                            start=(dc == 0), stop=False,
                        )
                        nc.tensor.matmul(
                            out=sc_half[:], lhsT=zh_c, rhs=elT_sb[:, dc, ks],
                            start=False, stop=False,
                        )
                    for dc in range(4):
                        zl_c = zlT[:, dc * P : (dc + 1) * P]
                        nc.tensor.matmul(
                            out=sc_half[:], lhsT=zl_c, rhs=ehT_sb[:, dc, ks],
                            start=False, stop=(dc == 3),
                        )
                    # negd half = fl(fl(|e_k|^2 + A_n) - 2c), reference rounding
                    nc.vector.scalar_tensor_tensor(
                        out=sc_sb[:, ks],
                        in0=bias_sb[:, ks],
                        scalar=zslots[:, t : t + 1],
                        in1=sc_half[:],
                        op0=mybir.AluOpType.add,
                        op1=mybir.AluOpType.subtract,
                    )
                    mh_t = mhpool.tile([P, 1], F32, tag="mh")
                    nc.vector.tensor_reduce(
                        out=mh_t[:], in_=sc_sb[:, ks],
                        axis=mybir.AxisListType.X, op=mybir.AluOpType.min,
                    )
                    mh[kh] = mh_t
                # row min across halves = |z_n - e_idx|^2, loss contribution
                nc.vector.tensor_tensor(
                    out=mslots[:, t : t + 1], in0=mh[0][:], in1=mh[1][:],
                    op=mybir.AluOpType.min,
                )
                # argmin: first position equal to the row min
                idx8 = ipool.tile([P, 8], U32)
                nc.vector.max_index(
                    out=idx8[:],
                    in_max=mslots[:, t : t + 1].to_broadcast([P, 8]),
                    in_values=sc_sb[:],
                )
                nc.gpsimd.tensor_copy(idxc[:, t : t + 1], idx8[:, 0:1])

                # gather z_q rows from the codebook in DRAM
                zq_sb = zqpool.tile([P, D], F32)
                nc.gpsimd.indirect_dma_start(
                    out=zq_sb[:],
                    out_offset=None,
                    in_=e_d[:],
                    in_offset=IndirectOffsetOnAxis(ap=idx8[:, 0:1], axis=0),
                )
                # faithful straight-through: out = fl(z + fl(z_q - z))
                dd_sb = zqpool.tile([P, D], F32)
                nc.gpsimd.tensor_tensor(
                    out=dd_sb[:], in0=zq_sb[:], in1=z_sb[:],
                    op=mybir.AluOpType.subtract,
                )
                st_sb = zqpool.tile([P, D], F32)
                nc.gpsimd.tensor_tensor(
                    out=st_sb[:], in0=z_sb[:], in1=dd_sb[:], op=mybir.AluOpType.add
                )
                nc.sync.dma_start(out=zq_d[t * P : (t + 1) * P, :], in_=st_sb[:])

            # ---- tail: loss partial + indices out (host sums partials) ----
            msum = cpool.tile([P, 1], F32)
            nc.vector.reduce_sum(msum[:], mslots[:], axis=mybir.AxisListType.X)
            nc.sync.dma_start(out=part_d[:], in_=msum[:])
            nc.sync.dma_start(out=idx_d[:], in_=idxc[:])

    nc.compile()
    return nc


_NC_CACHE = {}


def _get_nc(n_tiles):
    if n_tiles not in _NC_CACHE:
        _NC_CACHE[n_tiles] = build_core_program(n_tiles)
    return _NC_CACHE[n_tiles]


def kernel(z, embed_weight, _trace=False):
    z = np.asarray(z, dtype=np.float32)
    e = np.ascontiguousarray(np.asarray(embed_weight, dtype=np.float32))
    B, T, Dd = z.shape
    assert Dd == D and e.shape == (K, D)
    n = B * T
    zf = np.ascontiguousarray(z.reshape(n, D))
    rows = n // N_CORES
    n_tiles = rows // P

    import ml_dtypes

    eT2 = 2.0 * e.T.astype(np.float32)                 # [512, 1024], 2x exact
    ehT = eT2.astype(ml_dtypes.bfloat16)
    elT = (eT2 - ehT.astype(np.float32)).astype(ml_dtypes.bfloat16)
    ehT = np.ascontiguousarray(ehT)
    elT = np.ascontiguousarray(elT)
    bias_row = (e * e).sum(axis=1, dtype=np.float32)
    bias = np.ascontiguousarray(np.broadcast_to(bias_row[None, :], (P, K)))

    nc = _get_nc(n_tiles)
    in_maps = [
        {
            "z": zf[c * rows : (c + 1) * rows],
            "ehT": ehT,
            "elT": elT,
            "e": e,
            "bias": bias,
        }
        for c in range(N_CORES)
    ]
    res = bass_utils.run_bass_kernel_spmd(
        nc, in_maps, core_ids=list(range(N_CORES)), trace=_trace
    )

    zq = np.concatenate([res.results[c]["zq"] for c in range(N_CORES)], axis=0)
    z_q_st = zq.reshape(B, T, D)

    idx_parts = []
    for c in range(N_CORES):
        a = res.results[c]["idx"]                        # [128, n_tiles]
        idx_parts.append(np.ascontiguousarray(a.T).reshape(-1))  # row t*128+p
    indices = np.concatenate(idx_parts).astype(np.int32).reshape(B, T)

    total = np.sum(
        [res.results[c]["partial"].astype(np.float64).sum() for c in range(N_CORES)]
    )
    loss = np.float32(1.25 * total / (n * D))

    if _trace:
        kernel.last_exec_time_ns = res.exec_time_ns
        kernel.last_results = res
    return z_q_st, loss, indices


# revision 31
# speedup vs baseline: 2.2306x; 1.0221x over previous
"""VQ codebook-lookup kernel for Trainium2 (Bass/Tile), 8-core data-parallel.

Problem: z [16, 4096, 512] f32, codebook e [1024, 512] f32.
The reference computes d[n,k] = -|z_n|^2 - |e_k|^2 + 2 z_n.e_k in fp32, where
the |z|^2 term (~512) quantizes d at ulp(512) ~ 6e-5 — hundreds of rows have
true gaps below that, so argmax ties are resolved by this exact quantization.
We replicate the same fp32 arithmetic shape (verified bit-robust offline):
  negd[n,k] = fl( fl(|e_k|^2 + |z_n|^2) - (2 z_n).e_k )     (= -d)
  idx[n]    = argmin_k negd  (first occurrence, via max_index equality search)
  z_q       = e[idx]          (z_q_st == z_q numerically)
  loss      = 1.25 * (sum_n min_k negd) / (N * D)    since |z-e|^2 = -d
The factor 2 is folded into the codebook on the host (exact fp32 scaling).

Sharding: z split into 8 shards of 8192 rows (batch-dim data parallel),
codebook replicated. Per-core kernel computes its z_q shard, indices shard
and the scalar partial sum; host sums partials into the loss.
"""

import numpy as np

import concourse.bacc as bacc
import concourse.bass as bass
import concourse.mybir as mybir
import concourse.tile as tile
from concourse import bass_utils
from concourse.bass import IndirectOffsetOnAxis
from concourse.masks import make_identity

F32 = mybir.dt.float32
BF16 = mybir.dt.bfloat16
U32 = mybir.dt.uint32

P = 128          # partitions / rows per tile
D = 512          # embedding dim
K = 1024         # number of codes
N_CORES = 8
ROWS_PER_CORE = 8192


def build_core_program(n_tiles=ROWS_PER_CORE // P):
    """Bass program for one core's shard: [n_tiles*128, 512] rows of z."""
    rows = n_tiles * P
    nc = bacc.Bacc(None, target_bir_lowering=False)

    z_d = nc.dram_tensor("z", [rows, D], F32, kind="ExternalInput")
    ehT_d = nc.dram_tensor("ehT", [D, K], BF16, kind="ExternalInput")
    elT_d = nc.dram_tensor("elT", [D, K], BF16, kind="ExternalInput")
    e_d = nc.dram_tensor("e", [K, D], F32, kind="ExternalInput")
    bias_d = nc.dram_tensor("bias", [P, K], F32, kind="ExternalInput")

    zq_d = nc.dram_tensor("zq", [rows, D], F32, kind="ExternalOutput")
    idx_d = nc.dram_tensor("idx", [P, n_tiles], U32, kind="ExternalOutput")
    part_d = nc.dram_tensor("partial", [P, 1], F32, kind="ExternalOutput")

    with tile.TileContext(nc) as tc:
        with (
            tc.tile_pool(name="const", bufs=1) as cpool,
            tc.tile_pool(name="zin", bufs=6) as zpool,
            tc.tile_pool(name="zt", bufs=5) as ztpool,
            tc.tile_pool(name="sc", bufs=4) as scpool,
            tc.tile_pool(name="zq", bufs=4) as zqpool,
            tc.tile_pool(name="sq", bufs=3) as sqpool,
            tc.tile_pool(name="idx8", bufs=6) as ipool,
            tc.tile_pool(name="ps_zt", bufs=2, space="PSUM") as ps_zt,
            tc.tile_pool(name="ps_sc", bufs=3, space="PSUM") as ps_sc,
        ):
            # ---- one-time setup ----
            ident = cpool.tile([P, P], F32)
            make_identity(nc, ident[:])

            ehT_sb = cpool.tile([P, 4, K], BF16)   # [p, dchunk, k]
            nc.sync.dma_start(
                out=ehT_sb[:], in_=ehT_d[:].rearrange("(c p) k -> p c k", p=P)
            )
            elT_sb = cpool.tile([P, 4, K], BF16)
            nc.sync.dma_start(
                out=elT_sb[:], in_=elT_d[:].rearrange("(c p) k -> p c k", p=P)
            )
            bias_sb = cpool.tile([P, K], F32)
            nc.sync.dma_start(out=bias_sb[:], in_=bias_d[:])

            mslots = cpool.tile([P, n_tiles], F32)
            zslots = cpool.tile([P, n_tiles], F32)
            idxc = cpool.tile([P, n_tiles], U32)

            # ---- main loop over 128-row tiles ----
            for t in range(n_tiles):
                z_sb = zpool.tile([P, D], F32)
                nc.sync.dma_start(out=z_sb[:], in_=z_d[t * P : (t + 1) * P, :])

                # transpose z tile: zT[d, n] in 4 chunks of [128, 128]
                zT_ps = ps_zt.tile([P, D], F32, space="PSUM")
                for dc in range(4):
                    nc.tensor.transpose(
                        out=zT_ps[:, dc * P : (dc + 1) * P],
                        in_=z_sb[:, dc * P : (dc + 1) * P],
                        identity=ident[:],
                    )
                # bf16 hi/lo split of zT straight from PSUM (Dekker-style);
                # hi-cast on the (idle) scalar engine, residual on DVE
                zhT = ztpool.tile([P, D], BF16)
                nc.scalar.activation(
                    out=zhT[:], in_=zT_ps[:], func=mybir.ActivationFunctionType.Copy
                )
                zlT = ztpool.tile([P, D], BF16)
                nc.vector.tensor_tensor(
                    out=zlT[:], in0=zT_ps[:], in1=zhT[:], op=mybir.AluOpType.subtract
                )

                # row norms: zslots[:, t] = sum_d z^2 (per partition row)
                zsq = sqpool.tile([P, D], F32)
                nc.scalar.activation(
                    out=zsq[:],
                    in_=z_sb[:],
                    func=mybir.ActivationFunctionType.Square,
                    accum_out=zslots[:, t : t + 1],
                )

                # 2c = zh.ehT + zh.elT + zl.ehT (bf16x3), PSUM f32 accumulate.
                # All zh passes first (zh is ready one DVE op earlier than zl),
                # weights reused across kh to cut LDWEIGHTS pressure.
                sc_ps = ps_sc.tile([P, K], F32, space="PSUM")
                for dc in range(4):
                    zh_c = zhT[:, dc * P : (dc + 1) * P]
                    for kh in range(2):
                        ks = slice(kh * 512, (kh + 1) * 512)
                        nc.tensor.matmul(
                            out=sc_ps[:, ks], lhsT=zh_c, rhs=ehT_sb[:, dc, ks],
                            start=(dc == 0), stop=False,
                        )
                        nc.tensor.matmul(
                            out=sc_ps[:, ks], lhsT=zh_c, rhs=elT_sb[:, dc, ks],
                            start=False, stop=False,
                        )
                for dc in range(4):
                    zl_c = zlT[:, dc * P : (dc + 1) * P]
                    for kh in range(2):
                        ks = slice(kh * 512, (kh + 1) * 512)
                        nc.tensor.matmul(
                            out=sc_ps[:, ks], lhsT=zl_c, rhs=ehT_sb[:, dc, ks],
                            start=False, stop=(dc == 3),
                        )

                # negd = fl(fl(|e_k|^2 + A_n) - 2c)  — mimics reference rounding
                sc_sb = scpool.tile([P, K], F32)
                nc.vector.scalar_tensor_tensor(
                    out=sc_sb[:],
                    in0=bias_sb[:],
                    scalar=zslots[:, t : t + 1],
                    in1=sc_ps[:],
                    op0=mybir.AluOpType.add,
                    op1=mybir.AluOpType.subtract,
                )
                # row min = |z_n - e_idx|^2 (quantized), also loss contribution
                nc.vector.tensor_reduce(
                    out=mslots[:, t : t + 1],
                    in_=sc_sb[:],
                    axis=mybir.AxisListType.X,
                    op=mybir.AluOpType.min,
                )
                # argmin: first position equal to the row min
                idx8 = ipool.tile([P, 8], U32)
                nc.vector.max_index(
                    out=idx8[:],
                    in_max=mslots[:, t : t + 1].to_broadcast([P, 8]),
                    in_values=sc_sb[:],
                )
                nc.gpsimd.tensor_copy(idxc[:, t : t + 1], idx8[:, 0:1])

                # gather z_q rows from the codebook in DRAM
                zq_sb = zqpool.tile([P, D], F32)
                nc.gpsimd.indirect_dma_start(
                    out=zq_sb[:],
                    out_offset=None,
                    in_=e_d[:],
                    in_offset=IndirectOffsetOnAxis(ap=idx8[:, 0:1], axis=0),
                )
                # faithful straight-through: out = fl(z + fl(z_q - z))
                dd_sb = zqpool.tile([P, D], F32)
                nc.gpsimd.tensor_tensor(
                    out=dd_sb[:], in0=zq_sb[:], in1=z_sb[:],
                    op=mybir.AluOpType.subtract,
                )
                st_sb = zqpool.tile([P, D], F32)
                nc.gpsimd.tensor_tensor(
                    out=st_sb[:], in0=z_sb[:], in1=dd_sb[:], op=mybir.AluOpType.add
                )
                nc.sync.dma_start(out=zq_d[t * P : (t + 1) * P, :], in_=st_sb[:])

            # ---- tail: loss partial + indices out (host sums partials) ----
            msum = cpool.tile([P, 1], F32)
            nc.vector.reduce_sum(msum[:], mslots[:], axis=mybir.AxisListType.X)
            nc.sync.dma_start(out=part_d[:], in_=msum[:])
            nc.sync.dma_start(out=idx_d[:], in_=idxc[:])

    nc.compile()
    return nc


_NC_CACHE = {}


def _get_nc(n_tiles):
    if n_tiles not in _NC_CACHE:
        _NC_CACHE[n_tiles] = build_core_program(n_tiles)
    return _NC_CACHE[n_tiles]


def kernel(z, embed_weight, _trace=False):
    z = np.asarray(z, dtype=np.float32)
    e = np.ascontiguousarray(np.asarray(embed_weight, dtype=np.float32))
    B, T, Dd = z.shape
    assert Dd == D and e.shape == (K, D)
    n = B * T
    zf = np.ascontiguousarray(z.reshape(n, D))
    rows = n // N_CORES
    n_tiles = rows // P

    import ml_dtypes

    eT2 = 2.0 * e.T.astype(np.float32)                 # [512, 1024], 2x exact
    ehT = eT2.astype(ml_dtypes.bfloat16)
    elT = (eT2 - ehT.astype(np.float32)).astype(ml_dtypes.bfloat16)
    ehT = np.ascontiguousarray(ehT)
    elT = np.ascontiguousarray(elT)
    bias_row = (e * e).sum(axis=1, dtype=np.float32)
    bias = np.ascontiguousarray(np.broadcast_to(bias_row[None, :], (P, K)))

    nc = _get_nc(n_tiles)
    in_maps = [
        {
            "z": zf[c * rows : (c + 1) * rows],
            "ehT": ehT,
            "elT": elT,
            "e": e,
            "bias": bias,
        }
        for c in range(N_CORES)
    ]
    res = bass_utils.run_bass_kernel_spmd(
        nc, in_maps, core_ids=list(range(N_CORES)), trace=_trace
    )

    zq = np.concatenate([res.results[c]["zq"] for c in range(N_CORES)], axis=0)
    z_q_st = zq.reshape(B, T, D)

    idx_parts = []
    for c in range(N_CORES):
        a = res.results[c]["idx"]                        # [128, n_tiles]
        idx_parts.append(np.ascontiguousarray(a.T).reshape(-1))  # row t*128+p
    indices = np.concatenate(idx_parts).astype(np.int32).reshape(B, T)

    total = np.sum(
        [res.results[c]["partial"].astype(np.float64).sum() for c in range(N_CORES)]
    )
    loss = np.float32(1.25 * total / (n * D))

    if _trace:
        kernel.last_exec_time_ns = res.exec_time_ns
        kernel.last_results = res
    return z_q_st, loss, indices


# revision 35
# speedup vs baseline: 2.3021x; 1.0321x over previous
"""VQ codebook-lookup kernel for Trainium2 (Bass/Tile), 8-core data-parallel.

Problem: z [16, 4096, 512] f32, codebook e [1024, 512] f32.
The reference computes d[n,k] = -|z_n|^2 - |e_k|^2 + 2 z_n.e_k in fp32, where
the |z|^2 term (~512) quantizes d at ulp(512) ~ 6e-5 — hundreds of rows have
true gaps below that, so argmax ties are resolved by this exact quantization.
We replicate the same fp32 arithmetic shape (verified bit-robust offline):
  negd[n,k] = fl( fl(|e_k|^2 + |z_n|^2) - (2 z_n).e_k )     (= -d)
  idx[n]    = argmin_k negd  (first occurrence, via max_index equality search)
  z_q       = e[idx]          (z_q_st == z_q numerically)
  loss      = 1.25 * (sum_n min_k negd) / (N * D)    since |z-e|^2 = -d
The factor 2 is folded into the codebook on the host (exact fp32 scaling).

Sharding: z split into 8 shards of 8192 rows (batch-dim data parallel),
codebook replicated. Per-core kernel computes its z_q shard, indices shard
and the scalar partial sum; host sums partials into the loss.
"""

import numpy as np

import concourse.bacc as bacc
import concourse.bass as bass
import concourse.mybir as mybir
import concourse.tile as tile
from concourse import bass_utils
from concourse.bass import IndirectOffsetOnAxis
from concourse.masks import make_identity

F32 = mybir.dt.float32
BF16 = mybir.dt.bfloat16
U32 = mybir.dt.uint32

P = 128          # partitions / rows per tile
D = 512          # embedding dim
K = 1024         # number of codes
N_CORES = 8
ROWS_PER_CORE = 8192


def build_core_program(n_tiles=ROWS_PER_CORE // P):
    """Bass program for one core's shard: [n_tiles*128, 512] rows of z."""
    rows = n_tiles * P
    nc = bacc.Bacc(None, target_bir_lowering=False)

    z_d = nc.dram_tensor("z", [rows, D], F32, kind="ExternalInput")
    ehT_d = nc.dram_tensor("ehT", [D, K], BF16, kind="ExternalInput")
    elT_d = nc.dram_tensor("elT", [D, K], BF16, kind="ExternalInput")
    e_d = nc.dram_tensor("e", [K, D], F32, kind="ExternalInput")
    bias_d = nc.dram_tensor("bias", [P, K], F32, kind="ExternalInput")

    zq_d = nc.dram_tensor("zq", [rows, D], F32, kind="ExternalOutput")
    idx_d = nc.dram_tensor("idx", [P, n_tiles], U32, kind="ExternalOutput")
    part_d = nc.dram_tensor("partial", [P, 1], F32, kind="ExternalOutput")

    with tile.TileContext(nc) as tc:
        with (
            tc.tile_pool(name="const", bufs=1) as cpool,
            tc.tile_pool(name="zin", bufs=4) as zpool,
            tc.tile_pool(name="zt", bufs=3) as ztpool,
            tc.tile_pool(name="sc", bufs=3) as scpool,
            tc.tile_pool(name="zq", bufs=3) as zqpool,
            tc.tile_pool(name="sq", bufs=2) as sqpool,
            tc.tile_pool(name="idx8", bufs=4) as ipool,
            tc.tile_pool(name="zst", bufs=3) as zstpool,
            tc.tile_pool(name="ps_zt", bufs=2, space="PSUM") as ps_zt,
            tc.tile_pool(name="ps_sc", bufs=3, space="PSUM") as ps_sc,
        ):
            # ---- one-time setup ----
            ident = cpool.tile([P, P], F32)
            make_identity(nc, ident[:])

            ehT_sb = cpool.tile([P, 4, K], BF16)   # [p, dchunk, k]
            nc.sync.dma_start(
                out=ehT_sb[:], in_=ehT_d[:].rearrange("(c p) k -> p c k", p=P)
            )
            elT_sb = cpool.tile([P, 4, K], BF16)
            nc.sync.dma_start(
                out=elT_sb[:], in_=elT_d[:].rearrange("(c p) k -> p c k", p=P)
            )
            bias_sb = cpool.tile([P, K], F32)
            nc.sync.dma_start(out=bias_sb[:], in_=bias_d[:])

            mslots = cpool.tile([P, n_tiles], F32)
            zslots = cpool.tile([P, n_tiles], F32)
            idxc = cpool.tile([P, n_tiles], U32)

            # ---- main loop over 128-row tiles ----
            for t in range(n_tiles):
                z_sb = zpool.tile([P, D], F32)
                nc.sync.dma_start(out=z_sb[:], in_=z_d[t * P : (t + 1) * P, :])

                # early private copy of z for the straight-through tail, so
                # z_sb's slot frees as soon as the tile front-half is done
                # (the late gpsimd reads otherwise delay the next z load)
                zst = zstpool.tile([P, D], F32)
                nc.scalar.activation(
                    out=zst[:], in_=z_sb[:], func=mybir.ActivationFunctionType.Copy
                )

                # transpose z tile: zT[d, n] in 4 chunks of [128, 128]
                zT_ps = ps_zt.tile([P, D], F32, space="PSUM")
                for dc in range(4):
                    nc.tensor.transpose(
                        out=zT_ps[:, dc * P : (dc + 1) * P],
                        in_=z_sb[:, dc * P : (dc + 1) * P],
                        identity=ident[:],
                    )
                # bf16 hi/lo split of zT straight from PSUM (Dekker-style);
                # hi-cast on the (idle) scalar engine, residual on DVE
                zhT = ztpool.tile([P, D], BF16)
                nc.scalar.activation(
                    out=zhT[:], in_=zT_ps[:], func=mybir.ActivationFunctionType.Copy
                )
                zlT = ztpool.tile([P, D], BF16)
                nc.vector.tensor_tensor(
                    out=zlT[:], in0=zT_ps[:], in1=zhT[:], op=mybir.AluOpType.subtract
                )

                # row norms: zslots[:, t] = sum_d z^2 (per partition row)
                zsq = sqpool.tile([P, D], F32)
                nc.scalar.activation(
                    out=zsq[:],
                    in_=z_sb[:],
                    func=mybir.ActivationFunctionType.Square,
                    accum_out=zslots[:, t : t + 1],
                )

                # 2c = zh.ehT + zh.elT + zl.ehT (bf16x3), PSUM f32 accumulate.
                # All zh passes first (zh is ready one DVE op earlier than zl),
                # weights reused across kh to cut LDWEIGHTS pressure.
                sc_ps = ps_sc.tile([P, K], F32, space="PSUM")
                for dc in range(4):
                    zh_c = zhT[:, dc * P : (dc + 1) * P]
                    for kh in range(2):
                        ks = slice(kh * 512, (kh + 1) * 512)
                        nc.tensor.matmul(
                            out=sc_ps[:, ks], lhsT=zh_c, rhs=ehT_sb[:, dc, ks],
                            start=(dc == 0), stop=False,
                        )
                        nc.tensor.matmul(
                            out=sc_ps[:, ks], lhsT=zh_c, rhs=elT_sb[:, dc, ks],
                            start=False, stop=False,
                        )
                for dc in range(4):
                    zl_c = zlT[:, dc * P : (dc + 1) * P]
                    for kh in range(2):
                        ks = slice(kh * 512, (kh + 1) * 512)
                        nc.tensor.matmul(
                            out=sc_ps[:, ks], lhsT=zl_c, rhs=ehT_sb[:, dc, ks],
                            start=False, stop=(dc == 3),
                        )

                # negd = fl(fl(|e_k|^2 + A_n) - 2c)  — mimics reference rounding
                sc_sb = scpool.tile([P, K], F32)
                nc.vector.scalar_tensor_tensor(
                    out=sc_sb[:],
                    in0=bias_sb[:],
                    scalar=zslots[:, t : t + 1],
                    in1=sc_ps[:],
                    op0=mybir.AluOpType.add,
                    op1=mybir.AluOpType.subtract,
                )
                # row min = |z_n - e_idx|^2 (quantized), also loss contribution
                nc.vector.tensor_reduce(
                    out=mslots[:, t : t + 1],
                    in_=sc_sb[:],
                    axis=mybir.AxisListType.X,
                    op=mybir.AluOpType.min,
                )
                # argmin: first position equal to the row min
                idx8 = ipool.tile([P, 8], U32)
                nc.vector.max_index(
                    out=idx8[:],
                    in_max=mslots[:, t : t + 1].to_broadcast([P, 8]),
                    in_values=sc_sb[:],
                )
                nc.gpsimd.tensor_copy(idxc[:, t : t + 1], idx8[:, 0:1])

                # gather z_q rows from the codebook in DRAM
                zq_sb = zqpool.tile([P, D], F32)
                nc.gpsimd.indirect_dma_start(
                    out=zq_sb[:],
                    out_offset=None,
                    in_=e_d[:],
                    in_offset=IndirectOffsetOnAxis(ap=idx8[:, 0:1], axis=0),
                )
                # faithful straight-through: out = fl(z + fl(z_q - z))
                dd_sb = zqpool.tile([P, D], F32)
                nc.gpsimd.tensor_tensor(
                    out=dd_sb[:], in0=zq_sb[:], in1=zst[:],
                    op=mybir.AluOpType.subtract,
                )
                st_sb = zqpool.tile([P, D], F32)
                nc.gpsimd.tensor_tensor(
                    out=st_sb[:], in0=zst[:], in1=dd_sb[:], op=mybir.AluOpType.add
                )
                nc.sync.dma_start(out=zq_d[t * P : (t + 1) * P, :], in_=st_sb[:])

            # ---- tail: loss partial + indices out (host sums partials) ----
            msum = cpool.tile([P, 1], F32)
            nc.vector.reduce_sum(msum[:], mslots[:], axis=mybir.AxisListType.X)
            nc.sync.dma_start(out=part_d[:], in_=msum[:])
            nc.sync.dma_start(out=idx_d[:], in_=idxc[:])

    nc.compile()
    return nc


_NC_CACHE = {}


def _get_nc(n_tiles):
    if n_tiles not in _NC_CACHE:
        _NC_CACHE[n_tiles] = build_core_program(n_tiles)
    return _NC_CACHE[n_tiles]


def kernel(z, embed_weight, _trace=False):
    z = np.asarray(z, dtype=np.float32)
    e = np.ascontiguousarray(np.asarray(embed_weight, dtype=np.float32))
    B, T, Dd = z.shape
    assert Dd == D and e.shape == (K, D)
    n = B * T
    zf = np.ascontiguousarray(z.reshape(n, D))
    rows = n // N_CORES
    n_tiles = rows // P

    import ml_dtypes

    eT2 = 2.0 * e.T.astype(np.float32)                 # [512, 1024], 2x exact
    ehT = eT2.astype(ml_dtypes.bfloat16)
    elT = (eT2 - ehT.astype(np.float32)).astype(ml_dtypes.bfloat16)
    ehT = np.ascontiguousarray(ehT)
    elT = np.ascontiguousarray(elT)
    bias_row = (e * e).sum(axis=1, dtype=np.float32)
    bias = np.ascontiguousarray(np.broadcast_to(bias_row[None, :], (P, K)))

    nc = _get_nc(n_tiles)
    in_maps = [
        {
            "z": zf[c * rows : (c + 1) * rows],
            "ehT": ehT,
            "elT": elT,
            "e": e,
            "bias": bias,
        }
        for c in range(N_CORES)
    ]
    res = bass_utils.run_bass_kernel_spmd(
        nc, in_maps, core_ids=list(range(N_CORES)), trace=_trace
    )

    zq = np.concatenate([res.results[c]["zq"] for c in range(N_CORES)], axis=0)
    z_q_st = zq.reshape(B, T, D)

    idx_parts = []
    for c in range(N_CORES):
        a = res.results[c]["idx"]                        # [128, n_tiles]
        idx_parts.append(np.ascontiguousarray(a.T).reshape(-1))  # row t*128+p
    indices = np.concatenate(idx_parts).astype(np.int32).reshape(B, T)

    total = np.sum(
        [res.results[c]["partial"].astype(np.float64).sum() for c in range(N_CORES)]
    )
    loss = np.float32(1.25 * total / (n * D))

    if _trace:
        kernel.last_exec_time_ns = res.exec_time_ns
        kernel.last_results = res
    return z_q_st, loss, indices
